# revision 16
# baseline (speedup 1.0000x reference)
"""Trainium2 Bass kernel for DihedralToCartesian (NeRF-style dihedral->xyz chain).

Full-input contract: kernel(angles[65536,252], prev_three[65536,3,3]) -> [65536,126,3].
Batch sharded 8 ways (8192 rows/core, pure data parallelism).

Math (validated vs the JAX reference on the full real batch, rel err ~4e-3,
tolerance 2e-2): the reference's per-atom frame update reduces to
    h'   = cd*f2 - sd*f3        (cd,sd = damped cos/sin(theta), invg folded in)
    f1'  = -cosA*f1 - sinA*h'
    f2'  =  sinA*f1 - cosA*h'
    f3'  =  sd*f2 + cd*f3
    v_i  = bond*(cosA*f1 + sinA*h') = -bond*f1'
    d_i  = d_{i-1} + v_i
Substitutions that make the chain 4 DVE fp32 ops/atom with no tensor-coeff
work on the f1 side:
  f1m := -f1   ->  f1m' = -cosA*f1m + sinA*h'     (one stt, fcm=-cosA*f1m from ACT)
                   v_i  = bond * f1m'             (one ACT const-mul, off-path)
  f2s := sig*f2 (sig_i = sinA_{i-1}, folded into the coefficient planes)
               ->  f2s' = -cosA*f1m' - f1m        (one stt, no extra products)
Per atom on DVE: one fused [128,12,64] coefficient-product mul (5-dim
broadcast AP), one paired add producing (h', f3'), two stt.  Positions are
not in the chain: v staged fp16, cumsummed by masked tensor_tensor_scan per
42-atom chunk, DMA'd out fp16 (host converts to fp32).
"""

import os
import sys

import numpy as np

for _p in ("/opt/trn_rl_repo", os.path.expanduser("~/.axon_site/_ro/trn_rl_repo")):
    if os.path.isdir(_p) and _p not in sys.path:
        sys.path.insert(0, _p)

import concourse.bass as bass
import concourse.bacc as bacc
import concourse.mybir as mybir
import concourse.tile as tile
from concourse.bass_utils import run_bass_kernel_spmd

F32 = mybir.dt.float32
F16 = mybir.dt.float16
AOP = mybir.AluOpType
AF = mybir.ActivationFunctionType

N_CORES = 8
B_FULL = 65536
BS = B_FULL // N_CORES  # 8192 rows per core
N = 126                 # atoms
P = 128                 # partitions
J = BS // P             # 64 batch columns per partition
CH = 9                  # atoms per precompute chunk (14 chunks)
CV = 42                 # atoms per v/scan/output chunk (3 chunks)

_ALPHA = np.array([2.028, 2.124, 1.941], np.float64)
_BOND = np.array([1.329, 1.458, 1.523], np.float64)
_CA = np.cos(_ALPHA)
_SA = np.sin(_ALPHA)


def _emit(nc: bass.Bass):
    angles = nc.dram_tensor("angles", [BS, 2 * N], F32, kind="ExternalInput").ap()
    prev = nc.dram_tensor("prev_three", [BS, 3, 3], F32, kind="ExternalInput").ap()
    out = nc.dram_tensor("out", [BS, N, 3], F16, kind="ExternalOutput").ap()

    ang_r = angles.rearrange("(p j) c -> p j c", p=P)          # [128, 64, 252]
    prev_r = prev.rearrange("(p j) r c -> p j (r c)", p=P)     # [128, 64, 9]
    out_r = out.rearrange("(p j) a c -> p j (a c)", p=P)       # [128, 64, 378]

    with tile.TileContext(nc) as tc:
        with (
            tc.tile_pool(name="planes", bufs=1) as planes,
            tc.tile_pool(name="coeff", bufs=2) as coeffp,
            tc.tile_pool(name="vout", bufs=2) as voutp,
            tc.tile_pool(name="state", bufs=1) as state,
            tc.tile_pool(name="pre", bufs=2) as pre,
            tc.tile_pool(name="scratch", bufs=1) as scratch,
        ):
            rawS = planes.tile([P, J * N], F32, tag="rawS")
            rawC = planes.tile([P, J * N], F32, tag="rawC")
            pv = planes.tile([P, J * 9], F32, tag="pv")
            mask = planes.tile([P, J * CV], F16, tag="mask")
            isp = planes.tile([P, N], F32, tag="isp")  # 1/sig_i pattern

            nc.sync.dma_start(
                out=rawS[:].rearrange("p (j a) -> p j a", a=N), in_=ang_r[:, :, 0:N]
            )
            nc.sync.dma_start(
                out=rawC[:].rearrange("p (j a) -> p j a", a=N),
                in_=ang_r[:, :, N : 2 * N],
            )
            nc.sync.dma_start(out=pv[:].rearrange("p (j x) -> p j x", x=9), in_=prev_r)

            mview = mask[:].rearrange("p (j a) -> p j a", a=CV)
            nc.vector.memset(mask[:], 1.0)
            nc.vector.memset(mview[:, :, 0:1], 0.0)
            ispN = planes.tile([P, N], F32, tag="ispN")
            # ispP[i] = 1/sinA_{(i-1)%3} for i>=1, 1.0 for i=0; ispN = -ispP
            for k3 in range(3):
                v3 = isp[:].rearrange("p (a k) -> p a k", k=3)[:, :, k3]
                nc.vector.memset(v3, float(1.0 / _SA[(k3 - 1) % 3]))
                v3n = ispN[:].rearrange("p (a k) -> p a k", k=3)[:, :, k3]
                nc.vector.memset(v3n, float(-1.0 / _SA[(k3 - 1) % 3]))
            nc.vector.memset(isp[:, 0:1], 1.0)
            nc.vector.memset(ispN[:, 0:1], -1.0)

            # ---- chain state (all fp32) --------------------------------------
            # F: h'(0:3), f3(3:6), f2s(6:9); f1m = -f1, double-buffered.
            F = state.tile([P, 9, J], F32, tag="F")
            f1t = [
                state.tile([P, 3, J], F32, tag="f1a", name="f1a"),
                state.tile([P, 3, J], F32, tag="f1b", name="f1b"),
            ]
            U = state.tile([P, 12, J], F32, tag="U")
            fcm = state.tile([P, 3, J], F32, tag="fcm")
            vtmp = state.tile([P, 3, J], F16, tag="vtmp")

            # ---- initial frame from prev_three -------------------------------
            pv_r = pv[:].rearrange("p (j x) -> p x j", x=9)      # [128, 9, 64]
            a_ap = pv_r[:, 0:3, :]
            b_ap = pv_r[:, 3:6, :]
            c_ap = pv_r[:, 6:9, :]

            def cross(dst, x, y, eps):
                for c in range(3):
                    c1, c2 = (c + 1) % 3, (c + 2) % 3
                    m = scratch.tile([P, 1, J], F32, tag="cr_m")
                    qt = scratch.tile([P, 1, J], F32, tag="cr_q")
                    nc.vector.tensor_mul(m[:], x[:, c1 : c1 + 1, :], y[:, c2 : c2 + 1, :])
                    nc.vector.tensor_mul(qt[:], x[:, c2 : c2 + 1, :], y[:, c1 : c1 + 1, :])
                    nc.vector.scalar_tensor_tensor(
                        dst[:, c : c + 1, :], m[:], eps, qt[:], AOP.add, AOP.subtract
                    )

            def rsqrt3(dst, src3):
                sq = scratch.tile([P, 3, J], F32, tag="in_sq")
                nc.scalar.square(sq[:], src3[:])
                s1 = scratch.tile([P, J], F32, tag="in_s1")
                nc.vector.tensor_add(s1[:], sq[:, 0, :], sq[:, 1, :])
                s2_ = scratch.tile([P, J], F32, tag="in_s2")
                nc.vector.tensor_add(s2_[:], s1[:], sq[:, 2, :])
                lgi = scratch.tile([P, J], F32, tag="in_lg")
                nc.scalar.activation(lgi[:], s2_[:], AF.Ln)
                nc.scalar.activation(dst[:], lgi[:], AF.Exp, 0.0, -0.5)

            vv = scratch.tile([P, 3, J], F32, tag="in_v")
            nc.vector.scalar_tensor_tensor(
                vv[:], b_ap, 1e-8, c_ap, AOP.add, AOP.subtract
            )
            rv1 = scratch.tile([P, J], F32, tag="in_rv")
            rsqrt3(rv1, vv)
            f1_0 = scratch.tile([P, 3, J], F32, tag="in_f1")
            nc.vector.tensor_mul(
                f1_0[:], vv[:], rv1[:].unsqueeze(1).broadcast_to([P, 3, J])
            )
            nc.scalar.mul(f1t[0][:], f1_0[:], -1.0)  # f1m = -f1
            uu = scratch.tile([P, 3, J], F32, tag="in_u")
            nc.vector.tensor_sub(uu[:], b_ap, a_ap)
            ww = scratch.tile([P, 3, J], F32, tag="in_w")
            cross(ww, uu, f1_0, 1e-8)
            rw = scratch.tile([P, J], F32, tag="in_rw")
            rsqrt3(rw, ww)
            nc.vector.tensor_mul(
                F[:, 3:6, :], ww[:], rw[:].unsqueeze(1).broadcast_to([P, 3, J])
            )
            cross(F[:, 6:9, :], f1_0, F[:, 3:6, :], 0.0)  # f2sm(0) = -f2(0) = f1 x f3

            # ---- coefficient precompute (windowed, fp32) ---------------------
            c4_tiles = [None, None]

            def emit_chunk(q):
                asl = slice(CH * q, CH * (q + 1))
                rS = rawS[:].rearrange("p (j a) -> p j a", a=N)[:, :, asl]
                rC = rawC[:].rearrange("p (j a) -> p j a", a=N)[:, :, asl]
                SH = [P, J, CH]
                # coefficient layout [P, 4, CH, J]: innermost J contiguous for
                # the chain's per-atom reads; precompute writes are strided.
                c4 = coeffp.tile([P, 4, CH, J], F32, tag="c4", name=f"c4_{q}")
                c4_tiles[q % 2] = c4
                # r1 = rn*ig collapses to exactly 1/sqrt(s^2+c^2): the damped
                # normalize times the folded invg cancels the eps entirely.
                s2 = pre.tile(SH, F32, tag="p_a")
                nc.scalar.square(s2[:], rS)
                c2 = pre.tile(SH, F32, tag="p_b")
                nc.scalar.square(c2[:], rC)
                nn = pre.tile(SH, F32, tag="p_c")
                nc.gpsimd.tensor_add(nn[:], s2[:], c2[:])
                lnn = pre.tile(SH, F32, tag="p_a", name=f"lnn{q}")
                nc.scalar.activation(lnn[:], nn[:], AF.Ln)
                r1 = pre.tile(SH, F32, tag="p_b", name=f"r1{q}")
                nc.scalar.activation(r1[:], lnn[:], AF.Exp, 0.0, -0.5)
                r1p = pre.tile(SH, F32, tag="p_c", name=f"r1p{q}")
                ispP_b = isp[:, asl].unsqueeze(1).broadcast_to([P, J, CH])
                nc.gpsimd.tensor_mul(r1p[:], r1[:], ispP_b)
                r1n = pre.tile(SH, F32, tag="p_d")
                ispN_b = ispN[:, asl].unsqueeze(1).broadcast_to([P, J, CH])
                nc.gpsimd.tensor_mul(r1n[:], r1[:], ispN_b)
                # C4 entries: 0=sd, 1=cd2, 2=cd, 3=nsd2 (hm/f2sm sign flip)
                def cw(e):
                    return c4[:, e, :, :].rearrange("p a j -> p j a")

                nc.gpsimd.tensor_mul(cw(0), rS, r1[:])
                nc.gpsimd.tensor_mul(cw(1), rC, r1p[:])
                nc.gpsimd.tensor_mul(cw(2), rC, r1[:])
                nc.gpsimd.tensor_mul(cw(3), rS, r1n[:])

            emit_chunk(0)

            # ---- the chain ----------------------------------------------------
            v_tiles = [None, None]
            o_tiles = [None, None]
            # op1 input: (f3@3, f2sm@6) -> products (sd*f3, cd2*f2sm | cd*f3, nsd2*f2sm)
            f32v = F[:, 3:9, :].rearrange("p (c k) j -> p c k j", c=2)
            f32b = f32v.unsqueeze(1).broadcast_to([P, 2, 2, 3, J])
            uview = U[:].rearrange("p (r c k) j -> p r c k j", r=2, c=2)
            ug = U[:].rearrange("p (g k) j -> p g k j", g=4)
            hview = F[:, 0:3, :]  # hm = -h'

            for i in range(N):
                q, qa = divmod(i, CH)
                if qa == 0 and q + 1 < N // CH:
                    emit_chunk(q + 1)
                k, kv = divmod(i, CV)
                if kv == 0:
                    v_tiles[k % 2] = voutp.tile(
                        [P, J, CV, 3], F16, tag="vb", name=f"vb{k}"
                    )
                    o_tiles[k % 2] = voutp.tile(
                        [P, J, CV, 3], F16, tag="ob", name=f"ob{k}"
                    )
                vb = v_tiles[k % 2]
                p3 = i % 3
                ca, sa = float(_CA[p3]), float(_SA[p3])
                bond = float(_BOND[p3])
                f1c = f1t[i % 2]
                f1n = f1t[(i + 1) % 2]
                c4 = c4_tiles[q % 2]

                # fcm = -cosA*f1m, ready well before the stt needs it
                nc.gpsimd.tensor_scalar_mul(fcm[:], f1c[:], -ca)

                # fused fp32 products (coeff innermost-J contiguous)
                c4v = (
                    c4[:, :, qa, :]
                    .rearrange("p (r c) j -> p r c j", r=2)
                    .unsqueeze(3)
                    .broadcast_to([P, 2, 2, 3, J])
                )
                nc.vector.tensor_mul(uview, c4v, f32b)
                # (hm, f3') = pairwise sums -> F[0:6]
                nc.vector.tensor_add(
                    F[:, 0:6, :].rearrange("p (g k) j -> p g k j", g=2),
                    ug[:, 0:4:2, :, :],
                    ug[:, 1:4:2, :, :],
                )
                # f1m' = -sinA*hm + fcm
                nc.vector.scalar_tensor_tensor(
                    f1n[:], hview, -sa, fcm[:], AOP.mult, AOP.add
                )
                # f2sm' = cosA*f1m' + f1m
                nc.vector.scalar_tensor_tensor(
                    F[:, 6:9, :], f1n[:], ca, f1c[:], AOP.mult, AOP.add
                )
                # v_i = bond * f1m'  (ACT, off the critical path)
                if kv == 0:
                    vout = vtmp[:]
                else:
                    vout = vb[:, :, kv, :].rearrange("p j c -> p c j")
                nc.scalar.mul(vout, f1n[:], bond)
                if kv == 0:
                    if k == 0:
                        carry = c_ap.rearrange("p c j -> p j c")
                    else:
                        carry = o_tiles[(k + 1) % 2][:, :, CV - 1, :]
                    nc.gpsimd.tensor_add(
                        vb[:, :, 0, :], vtmp[:].rearrange("p c j -> p j c"), carry
                    )

                if kv == CV - 1:
                    ob = o_tiles[k % 2]
                    for c in range(3):
                        vvw = vb[:].rearrange("p j a c -> p c (j a)")[:, c, :]
                        oow = ob[:].rearrange("p j a c -> p c (j a)")[:, c, :]
                        nc.vector.tensor_tensor_scan(
                            oow, mask[:], vvw, 0.0, AOP.mult, AOP.add
                        )
                    nc.sync.dma_start(
                        out=out_r[:, :, 3 * CV * k : 3 * CV * (k + 1)],
                        in_=ob[:].rearrange("p j a c -> p j (a c)"),
                    )
    return nc


_NC_CACHE: dict = {}


def _get_nc():
    if "nc" not in _NC_CACHE:
        nc = bacc.Bacc("TRN2", target_bir_lowering=False, debug=False)
        _emit(nc)
        nc.compile()
        _NC_CACHE["nc"] = nc
    return _NC_CACHE["nc"]


def run_sharded(angles: np.ndarray, prev_three: np.ndarray, **kw):
    """Shard inputs over 8 cores, run, return BassKernelResults."""
    angles = np.ascontiguousarray(angles, np.float32)
    prev_three = np.ascontiguousarray(prev_three, np.float32)
    assert angles.shape == (B_FULL, 2 * N) and prev_three.shape == (B_FULL, 3, 3)
    in_maps = [
        {
            "angles": angles[i * BS : (i + 1) * BS],
            "prev_three": prev_three[i * BS : (i + 1) * BS],
        }
        for i in range(N_CORES)
    ]
    return run_bass_kernel_spmd(_get_nc(), in_maps, core_ids=list(range(N_CORES)), **kw)


def kernel(angles: np.ndarray, prev_three: np.ndarray) -> np.ndarray:
    res = run_sharded(angles, prev_three)
    return np.concatenate([r["out"] for r in res.results], axis=0).astype(np.float32)


# revision 19
# speedup vs baseline: 1.3846x; 1.3846x over previous
"""Trainium2 Bass kernel for DihedralToCartesian (NeRF-style dihedral->xyz chain).

Full-input contract: kernel(angles[65536,252], prev_three[65536,3,3]) -> [65536,126,3].
Batch sharded 8 ways (8192 rows/core, pure data parallelism).

Math (validated vs the JAX reference on the full real batch, rel err ~4e-3,
tolerance 2e-2): the reference's per-atom frame update reduces to
    h'   = cd*f2 - sd*f3        (cd,sd = damped cos/sin(theta), invg folded in)
    f1'  = -cosA*f1 - sinA*h'
    f2'  =  sinA*f1 - cosA*h'
    f3'  =  sd*f2 + cd*f3
    v_i  = bond*(cosA*f1 + sinA*h') = -bond*f1'
    d_i  = d_{i-1} + v_i
Substitutions that make the chain 4 DVE fp32 ops/atom with no tensor-coeff
work on the f1 side:
  f1m := -f1   ->  f1m' = -cosA*f1m + sinA*h'     (one stt, fcm=-cosA*f1m from ACT)
                   v_i  = bond * f1m'             (one ACT const-mul, off-path)
  f2s := sig*f2 (sig_i = sinA_{i-1}, folded into the coefficient planes)
               ->  f2s' = -cosA*f1m' - f1m        (one stt, no extra products)
Per atom on DVE: one fused [128,12,64] coefficient-product mul (5-dim
broadcast AP), one paired add producing (h', f3'), two stt.  Positions are
not in the chain: v staged fp16, cumsummed by masked tensor_tensor_scan per
42-atom chunk, DMA'd out fp16 (host converts to fp32).
"""

import os
import sys

import numpy as np

for _p in ("/opt/trn_rl_repo", os.path.expanduser("~/.axon_site/_ro/trn_rl_repo")):
    if os.path.isdir(_p) and _p not in sys.path:
        sys.path.insert(0, _p)

import concourse.bass as bass
import concourse.bacc as bacc
import concourse.mybir as mybir
import concourse.tile as tile
from concourse.bass_utils import run_bass_kernel_spmd

F32 = mybir.dt.float32
F16 = mybir.dt.float16
AOP = mybir.AluOpType
AF = mybir.ActivationFunctionType

N_CORES = 8
B_FULL = 65536
BS = B_FULL // N_CORES  # 8192 rows per core
N = 126                 # atoms
P = 128                 # partitions
J = BS // P             # 64 batch columns per partition
CH = 9                  # atoms per precompute chunk (14 chunks)
CV = 42                 # atoms per v/scan/output chunk (3 chunks)

_ALPHA = np.array([2.028, 2.124, 1.941], np.float64)
_BOND = np.array([1.329, 1.458, 1.523], np.float64)
_CA = np.cos(_ALPHA)
_SA = np.sin(_ALPHA)


def _emit(nc: bass.Bass):
    angles = nc.dram_tensor("angles", [BS, 2 * N], F32, kind="ExternalInput").ap()
    prev = nc.dram_tensor("prev_three", [BS, 3, 3], F32, kind="ExternalInput").ap()
    out = nc.dram_tensor("out", [BS, N, 3], F16, kind="ExternalOutput").ap()

    ang_r = angles.rearrange("(p j) c -> p j c", p=P)          # [128, 64, 252]
    prev_r = prev.rearrange("(p j) r c -> p j (r c)", p=P)     # [128, 64, 9]
    out_r = out.rearrange("(p j) a c -> p j (a c)", p=P)       # [128, 64, 378]

    with tile.TileContext(nc) as tc:
        with (
            tc.tile_pool(name="planes", bufs=1) as planes,
            tc.tile_pool(name="coeff", bufs=2) as coeffp,
            tc.tile_pool(name="vout", bufs=2) as voutp,
            tc.tile_pool(name="state", bufs=1) as state,
            tc.tile_pool(name="pre", bufs=2) as pre,
            tc.tile_pool(name="scratch", bufs=1) as scratch,
        ):
            rawS = planes.tile([P, J * N], F32, tag="rawS")
            rawC = planes.tile([P, J * N], F32, tag="rawC")
            pv = planes.tile([P, J * 9], F32, tag="pv")
            mask = planes.tile([P, J * CV], F16, tag="mask")
            isp = planes.tile([P, N], F32, tag="isp")  # 1/sig_i pattern

            nc.sync.dma_start(
                out=rawS[:].rearrange("p (j a) -> p j a", a=N), in_=ang_r[:, :, 0:N]
            )
            nc.sync.dma_start(
                out=rawC[:].rearrange("p (j a) -> p j a", a=N),
                in_=ang_r[:, :, N : 2 * N],
            )
            nc.sync.dma_start(out=pv[:].rearrange("p (j x) -> p j x", x=9), in_=prev_r)

            mview = mask[:].rearrange("p (j a) -> p j a", a=CV)
            nc.vector.memset(mask[:], 1.0)
            nc.vector.memset(mview[:, :, 0:1], 0.0)
            ispN = planes.tile([P, N], F32, tag="ispN")
            # ispP[i] = 1/sinA_{(i-1)%3} for i>=1, 1.0 for i=0; ispN = -ispP
            for k3 in range(3):
                v3 = isp[:].rearrange("p (a k) -> p a k", k=3)[:, :, k3]
                nc.vector.memset(v3, float(1.0 / _SA[(k3 - 1) % 3]))
                v3n = ispN[:].rearrange("p (a k) -> p a k", k=3)[:, :, k3]
                nc.vector.memset(v3n, float(-1.0 / _SA[(k3 - 1) % 3]))
            nc.vector.memset(isp[:, 0:1], 1.0)
            nc.vector.memset(ispN[:, 0:1], -1.0)

            # ---- chain state (all fp32) --------------------------------------
            # F: h'(0:3), f3(3:6), f2s(6:9); f1m = -f1, double-buffered.
            F = state.tile([P, 9, J], F32, tag="F")
            f1t = [
                state.tile([P, 3, J], F32, tag="f1a", name="f1a"),
                state.tile([P, 3, J], F32, tag="f1b", name="f1b"),
            ]
            U = state.tile([P, 12, J], F32, tag="U")
            fcm = state.tile([P, 3, J], F32, tag="fcm")
            vtmp = state.tile([P, 3, J], F16, tag="vtmp")

            # ---- initial frame from prev_three -------------------------------
            pv_r = pv[:].rearrange("p (j x) -> p x j", x=9)      # [128, 9, 64]
            a_ap = pv_r[:, 0:3, :]
            b_ap = pv_r[:, 3:6, :]
            c_ap = pv_r[:, 6:9, :]

            def cross(dst, x, y, eps):
                for c in range(3):
                    c1, c2 = (c + 1) % 3, (c + 2) % 3
                    m = scratch.tile([P, 1, J], F32, tag="cr_m")
                    qt = scratch.tile([P, 1, J], F32, tag="cr_q")
                    nc.vector.tensor_mul(m[:], x[:, c1 : c1 + 1, :], y[:, c2 : c2 + 1, :])
                    nc.vector.tensor_mul(qt[:], x[:, c2 : c2 + 1, :], y[:, c1 : c1 + 1, :])
                    nc.vector.scalar_tensor_tensor(
                        dst[:, c : c + 1, :], m[:], eps, qt[:], AOP.add, AOP.subtract
                    )

            def rsqrt3(dst, src3):
                sq = scratch.tile([P, 3, J], F32, tag="in_sq")
                nc.scalar.square(sq[:], src3[:])
                s1 = scratch.tile([P, J], F32, tag="in_s1")
                nc.vector.tensor_add(s1[:], sq[:, 0, :], sq[:, 1, :])
                s2_ = scratch.tile([P, J], F32, tag="in_s2")
                nc.vector.tensor_add(s2_[:], s1[:], sq[:, 2, :])
                lgi = scratch.tile([P, J], F32, tag="in_lg")
                nc.scalar.activation(lgi[:], s2_[:], AF.Ln)
                nc.scalar.activation(dst[:], lgi[:], AF.Exp, 0.0, -0.5)

            vv = scratch.tile([P, 3, J], F32, tag="in_v")
            nc.vector.scalar_tensor_tensor(
                vv[:], b_ap, 1e-8, c_ap, AOP.add, AOP.subtract
            )
            rv1 = scratch.tile([P, J], F32, tag="in_rv")
            rsqrt3(rv1, vv)
            f1_0 = scratch.tile([P, 3, J], F32, tag="in_f1")
            nc.vector.tensor_mul(
                f1_0[:], vv[:], rv1[:].unsqueeze(1).broadcast_to([P, 3, J])
            )
            nc.scalar.mul(f1t[0][:], f1_0[:], -1.0)  # f1m = -f1
            uu = scratch.tile([P, 3, J], F32, tag="in_u")
            nc.vector.tensor_sub(uu[:], b_ap, a_ap)
            ww = scratch.tile([P, 3, J], F32, tag="in_w")
            cross(ww, uu, f1_0, 1e-8)
            rw = scratch.tile([P, J], F32, tag="in_rw")
            rsqrt3(rw, ww)
            nc.vector.tensor_mul(
                F[:, 3:6, :], ww[:], rw[:].unsqueeze(1).broadcast_to([P, 3, J])
            )
            cross(F[:, 6:9, :], f1_0, F[:, 3:6, :], 0.0)  # f2sm(0) = -f2(0) = f1 x f3

            # ---- coefficient precompute (windowed, fp32) ---------------------
            c4_tiles = [None, None]

            def emit_chunk(q):
                asl = slice(CH * q, CH * (q + 1))
                rS = rawS[:].rearrange("p (j a) -> p j a", a=N)[:, :, asl]
                rC = rawC[:].rearrange("p (j a) -> p j a", a=N)[:, :, asl]
                SH = [P, J, CH]
                # coefficient layout [P, CH, 4, J]: each atom's [P,4,J] slice
                # is fully contiguous for the chain; precompute writes strided.
                c4 = coeffp.tile([P, CH, 4, J], F32, tag="c4", name=f"c4_{q}")
                c4_tiles[q % 2] = c4
                # r1 = rn*ig collapses to exactly 1/sqrt(s^2+c^2): the damped
                # normalize times the folded invg cancels the eps entirely.
                s2 = pre.tile(SH, F32, tag="p_a")
                nc.scalar.square(s2[:], rS)
                c2 = pre.tile(SH, F32, tag="p_b")
                nc.scalar.square(c2[:], rC)
                nn = pre.tile(SH, F32, tag="p_c")
                nc.gpsimd.tensor_add(nn[:], s2[:], c2[:])
                lnn = pre.tile(SH, F32, tag="p_a", name=f"lnn{q}")
                nc.scalar.activation(lnn[:], nn[:], AF.Ln)
                r1 = pre.tile(SH, F32, tag="p_b", name=f"r1{q}")
                nc.scalar.activation(r1[:], lnn[:], AF.Exp, 0.0, -0.5)
                r1p = pre.tile(SH, F32, tag="p_c", name=f"r1p{q}")
                ispP_b = isp[:, asl].unsqueeze(1).broadcast_to([P, J, CH])
                nc.gpsimd.tensor_mul(r1p[:], r1[:], ispP_b)
                r1n = pre.tile(SH, F32, tag="p_d")
                ispN_b = ispN[:, asl].unsqueeze(1).broadcast_to([P, J, CH])
                nc.gpsimd.tensor_mul(r1n[:], r1[:], ispN_b)
                # C4 entries: 0=sd, 1=cd2, 2=cd, 3=nsd2 (hm/f2sm sign flip)
                def cw(e):
                    return c4[:, :, e, :].rearrange("p a j -> p j a")

                nc.gpsimd.tensor_mul(cw(0), rS, r1[:])
                nc.gpsimd.tensor_mul(cw(1), rC, r1p[:])
                nc.gpsimd.tensor_mul(cw(2), rC, r1[:])
                nc.gpsimd.tensor_mul(cw(3), rS, r1n[:])

            emit_chunk(0)

            # ---- the chain ----------------------------------------------------
            v_tiles = [None, None]
            o_tiles = [None, None]
            # op1 input: (f3@3, f2sm@6) -> products (sd*f3, cd2*f2sm | cd*f3, nsd2*f2sm)
            f32v = F[:, 3:9, :].rearrange("p (c k) j -> p c k j", c=2)
            f32b = f32v.unsqueeze(1).broadcast_to([P, 2, 2, 3, J])
            uview = U[:].rearrange("p (r c k) j -> p r c k j", r=2, c=2)
            ug = U[:].rearrange("p (g k) j -> p g k j", g=4)
            hview = F[:, 0:3, :]  # hm = -h'

            for i in range(N):
                q, qa = divmod(i, CH)
                if qa == 0 and q + 1 < N // CH:
                    emit_chunk(q + 1)
                k, kv = divmod(i, CV)
                if kv == 0:
                    v_tiles[k % 2] = voutp.tile(
                        [P, J, CV, 3], F16, tag="vb", name=f"vb{k}"
                    )
                    o_tiles[k % 2] = voutp.tile(
                        [P, J, CV, 3], F16, tag="ob", name=f"ob{k}"
                    )
                vb = v_tiles[k % 2]
                p3 = i % 3
                ca, sa = float(_CA[p3]), float(_SA[p3])
                bond = float(_BOND[p3])
                f1c = f1t[i % 2]
                f1n = f1t[(i + 1) % 2]
                c4 = c4_tiles[q % 2]

                # fcm = -cosA*f1m, ready well before the stt needs it
                nc.scalar.mul(fcm[:], f1c[:], -ca)

                # fused fp32 products (coeff slice contiguous [P,4,J])
                c4v = (
                    c4[:, qa, :, :]
                    .rearrange("p (r c) j -> p r c j", r=2)
                    .unsqueeze(3)
                    .broadcast_to([P, 2, 2, 3, J])
                )
                nc.vector.tensor_mul(uview, c4v, f32b)
                # (hm, f3') = pairwise sums -> F[0:6]
                nc.vector.tensor_add(
                    F[:, 0:6, :].rearrange("p (g k) j -> p g k j", g=2),
                    ug[:, 0:4:2, :, :],
                    ug[:, 1:4:2, :, :],
                )
                # f1m' = -sinA*hm + fcm
                nc.vector.scalar_tensor_tensor(
                    f1n[:], hview, -sa, fcm[:], AOP.mult, AOP.add
                )
                # f2sm' = cosA*f1m' + f1m
                nc.vector.scalar_tensor_tensor(
                    F[:, 6:9, :], f1n[:], ca, f1c[:], AOP.mult, AOP.add
                )
                # v_i = bond * f1m'  (ACT, off the critical path)
                if kv == 0:
                    vout = vtmp[:]
                else:
                    vout = vb[:, :, kv, :].rearrange("p j c -> p c j")
                nc.scalar.mul(vout, f1n[:], bond)
                if kv == 0:
                    if k == 0:
                        carry = c_ap.rearrange("p c j -> p j c")
                    else:
                        carry = o_tiles[(k + 1) % 2][:, :, CV - 1, :]
                    nc.gpsimd.tensor_add(
                        vb[:, :, 0, :], vtmp[:].rearrange("p c j -> p j c"), carry
                    )

                if kv == CV - 1:
                    ob = o_tiles[k % 2]
                    for c in range(3):
                        vvw = vb[:].rearrange("p j a c -> p c (j a)")[:, c, :]
                        oow = ob[:].rearrange("p j a c -> p c (j a)")[:, c, :]
                        nc.vector.tensor_tensor_scan(
                            oow, mask[:], vvw, 0.0, AOP.mult, AOP.add
                        )
                    nc.sync.dma_start(
                        out=out_r[:, :, 3 * CV * k : 3 * CV * (k + 1)],
                        in_=ob[:].rearrange("p j a c -> p j (a c)"),
                    )
    return nc


_NC_CACHE: dict = {}


def _get_nc():
    if "nc" not in _NC_CACHE:
        nc = bacc.Bacc("TRN2", target_bir_lowering=False, debug=False)
        _emit(nc)
        nc.compile()
        _NC_CACHE["nc"] = nc
    return _NC_CACHE["nc"]


def run_sharded(angles: np.ndarray, prev_three: np.ndarray, **kw):
    """Shard inputs over 8 cores, run, return BassKernelResults."""
    angles = np.ascontiguousarray(angles, np.float32)
    prev_three = np.ascontiguousarray(prev_three, np.float32)
    assert angles.shape == (B_FULL, 2 * N) and prev_three.shape == (B_FULL, 3, 3)
    in_maps = [
        {
            "angles": angles[i * BS : (i + 1) * BS],
            "prev_three": prev_three[i * BS : (i + 1) * BS],
        }
        for i in range(N_CORES)
    ]
    return run_bass_kernel_spmd(_get_nc(), in_maps, core_ids=list(range(N_CORES)), **kw)


def kernel(angles: np.ndarray, prev_three: np.ndarray) -> np.ndarray:
    res = run_sharded(angles, prev_three)
    return np.concatenate([r["out"] for r in res.results], axis=0).astype(np.float32)
